# revision 1
# baseline (speedup 1.0000x reference)
"""DiceCE-with-ignore-index loss kernel for Trainium2, 8 NeuronCores.

Contract: kernel(logits, target) -> np.float32 scalar loss, matching
reference: CE (mean over valid voxels) + masked soft Dice (batch dice,
background excluded), ignore_index = -1.

Strategy
--------
Data-parallel over (b, d): 2*64 = 128 (b,d) slices -> 16 per core.
Host casts logits/target to bf16 (target values {-1..3} are exact).
Per core the kernel reduces its 1,048,576 voxels to partial sums; the
tiny (C,)-vector final combine happens on host in float64.

Per-voxel math (no max-subtraction: |x| <= ~6 for randn inputs):
  e_c = exp(x_c)                       ScalarE (one table load: Exp+Ln
  L   = log s,  r = exp(-L) = 1/s      forced into the combined set)
  s   = sum_c e_c                      VectorE adds (bf16 2x mode)
  vf  = (t>=0); w = vf*r               VectorE TS(4x) + TT(2x)
  z_c = e_c*w                          VectorE TT (2x)
  eq_c = (t==c)                        VectorE TS (4x)
  ca  = (t>=0)*L (+accum)              VectorE scalar_tensor_tensor
TensorE computes every masked sum via PSUM-accumulated "diag trick"
matmuls: per class, stationary = eq_c chunk, moving = packed
[z_c | x_c | ones] planes; trace(block0) = intersect[c],
trace(block1) = sum eq_c*x_c, block2 column = gt_sum[c].  A fourth diag
(stationary = w chunk, moving = packed [e_0..e_3] planes) yields
p_sum[c] = trace(block c) and count = sum_c p_sum[c].  Class-0 CE term
uses stationary eq_0 against x_0.  The extra PE work is deliberate: it
keeps warm-clock TensorE as busy as VectorE so the HAM clock gate never
sees an idle window and the 2.4 GHz clock holds.
CE = (sum vf*L - sum_c sum eq_c*x_c) / count.
"""
import os
import sys
from contextlib import ExitStack

for _p in ("/opt/trn_rl_repo", "/root/.axon_site/_ro/trn_rl_repo", "/root/.axon_site"):
    if os.path.isdir(_p) and _p not in sys.path:
        sys.path.append(_p)

import numpy as np
import ml_dtypes

import concourse.bass as bass
import concourse.tile as tile
from concourse import bacc, mybir
from concourse.bass_utils import run_bass_kernel_spmd

BF16 = mybir.dt.bfloat16
F32 = mybir.dt.float32
ALU = mybir.AluOpType
ACTF = mybir.ActivationFunctionType

P = 128          # partitions
FD = 1024        # free dim per megatile (small so PE idle gaps stay under
                 # the ~3.4us HAM window and TensorE holds its 2.4 GHz clock)
NMT = 8          # megatiles per core (8 * 128 * 1024 = 1,048,576 voxels)
NCHUNK = FD // P # 16 diag chunks per megatile
NCORES = 8
C = 4            # classes

B, D, H, W = 2, 64, 256, 256
SMOOTH_NR = 1e-05
SMOOTH_DR = 1e-05

_NC_CACHE = {}


def _patch_act_tables():
    """Force Exp and Ln to resolve to the combined natural_log_exp set so the
    kernel needs a single ACT_TABLE_LOAD instead of thrashing between the
    exp-only and ln-only sets every megatile."""
    import concourse.hw_specs as hw_specs
    if getattr(bacc, "_act_tables_patched", False):
        return
    orig = hw_specs.get_activation_tables

    def patched(arch):
        tables = {k: set(v) for k, v in orig(arch).items()}
        if "natural_log_exp_and_others" in tables:
            for name, fns in tables.items():
                if name != "natural_log_exp_and_others":
                    fns.discard(ACTF.Exp)
                    fns.discard(ACTF.Ln)
        return tables

    hw_specs.get_activation_tables = patched
    bacc.get_activation_tables = patched
    bacc._act_tables_patched = True


def _build_nc():
    _patch_act_tables()
    nc = bacc.Bacc("TRN2", target_bir_lowering=False, debug=False)

    X = nc.dram_tensor("x", [C, NMT, P, FD], BF16, kind="ExternalInput")
    T = nc.dram_tensor("t", [NMT, P, FD], BF16, kind="ExternalInput")
    # acc columns per megatile: [ca]
    OUT_ACC = nc.dram_tensor("out_acc", [P, NMT], F32, kind="ExternalOutput")
    # per class c=1..3: [z-diag 128 | x-diag 128 | ones-cols 128] = 384
    # class 0 x-diag: 128;  ps4 (w x e_c diags): 512
    OUT_PS = nc.dram_tensor("out_ps", [P, 3 * 384 + 128 + 512], F32, kind="ExternalOutput")

    with tile.TileContext(nc) as tc, ExitStack() as ctx:
        io = ctx.enter_context(tc.tile_pool(name="io", bufs=3))
        mid = ctx.enter_context(tc.tile_pool(name="mid", bufs=3))
        one = ctx.enter_context(tc.tile_pool(name="one", bufs=1))
        psum = ctx.enter_context(tc.tile_pool(name="psum", bufs=1, space="PSUM"))

        acc = one.tile([P, NMT], F32)
        ps = [psum.tile([P, 384], F32, name=f"ps{c}") for c in (1, 2, 3)]
        ps0 = psum.tile([P, 128], F32)
        ps4 = psum.tile([P, 512], F32)

        # manual triple buffers with a persistent ones-plane (plane 2)
        NBUF = 3
        zxbuf = [[one.tile([P, 3, FD], BF16, name=f"zx{c}_{ab}") for c in (1, 2, 3)]
                 for ab in range(NBUF)]
        x0buf = [one.tile([P, FD], BF16, name=f"x0_{ab}") for ab in range(NBUF)]
        for ab in range(NBUF):
            for z in zxbuf[ab]:
                nc.gpsimd.memset(z[:, 2, :], 1.0)

        for mt in range(NMT):
            zx = zxbuf[mt % NBUF]
            x0 = x0buf[mt % NBUF]
            t_sb = io.tile([P, FD], BF16, tag="t", name=f"t_{mt}")
            # all input DMAs on the sync queue: the gpsimd queue is busy with
            # the ones-plane memsets at t=0 and must not delay mt0's loads
            nc.sync.dma_start(t_sb[:], T[mt])
            nc.sync.dma_start(x0[:], X[0, mt])
            for i, c in enumerate((1, 2, 3)):
                nc.sync.dma_start(zx[i][:, 1, :], X[c, mt])

            # ---- VectorE masks first (only need t) so PE can start early ----
            eq = [mid.tile([P, FD], BF16, tag=f"eq{c}", name=f"eq{c}_{mt}") for c in range(C)]
            for c in range(C):
                nc.vector.tensor_scalar(
                    out=eq[c][:], in0=t_sb[:], scalar1=float(c), scalar2=None,
                    op0=ALU.is_equal)
            vf = one.tile([P, FD], BF16, name="vf")
            nc.vector.tensor_scalar(out=vf[:], in0=t_sb[:], scalar1=0.0, scalar2=None,
                                    op0=ALU.is_ge)

            # ---- TensorE: class-0 CE diag (ready early) ----
            first = mt == 0
            last = mt == NMT - 1
            for k in range(NCHUNK):
                sl = slice(k * P, (k + 1) * P)
                nc.tensor.matmul(
                    ps0[:], eq[0][:, sl], x0[:, sl],
                    start=(first and k == 0), stop=(last and k == NCHUNK - 1))

            # ---- ScalarE: exponentials into packed E planes ----
            E = mid.tile([P, C, FD], BF16, tag="E", name=f"E_{mt}")
            nc.scalar.activation(E[:, 0, :], x0[:], ACTF.Exp)
            for i in range(3):
                nc.scalar.activation(E[:, i + 1, :], zx[i][:, 1, :], ACTF.Exp)

            # ---- VectorE: s = e0+e1+e2+e3 (bf16 TT = 2x mode) ----
            s01 = one.tile([P, FD], BF16, name=f"s01")
            s23 = one.tile([P, FD], BF16, name=f"s23")
            s = one.tile([P, FD], BF16, name=f"s")
            nc.vector.tensor_add(s01[:], E[:, 0, :], E[:, 1, :])
            nc.vector.tensor_add(s23[:], E[:, 2, :], E[:, 3, :])
            nc.vector.tensor_add(s[:], s01[:], s23[:])

            # ---- ScalarE: L = log s ; r = exp(-L) ----
            L = mid.tile([P, FD], BF16, tag="L", name=f"L_{mt}")
            r = mid.tile([P, FD], BF16, tag="r", name=f"r_{mt}")
            nc.scalar.activation(L[:], s[:], ACTF.Ln)
            nc.scalar.activation(r[:], L[:], ACTF.Exp, scale=-1.0)

            # ---- VectorE products ----
            w = one.tile([P, FD], BF16, name="w")
            nc.vector.tensor_mul(w[:], vf[:], r[:])
            for i, c in enumerate((1, 2, 3)):
                nc.vector.tensor_mul(zx[i][:, 0, :], E[:, c, :], w[:])
            nc.vector.scalar_tensor_tensor(
                out=s23[:], in0=t_sb[:], scalar=0.0, in1=L[:],
                op0=ALU.is_ge, op1=ALU.mult,
                accum_out=acc[:, mt: mt + 1])

            # ---- TensorE: diag-trick accumulation ----
            for k in range(NCHUNK):
                sl = slice(k * P, (k + 1) * P)
                nc.tensor.matmul(
                    ps4[:], w[:, sl], E[:, :, sl],
                    start=(first and k == 0), stop=(last and k == NCHUNK - 1))
                for i in range(3):
                    nc.tensor.matmul(
                        ps[i][:], eq[i + 1][:, sl], zx[i][:, :, sl],
                        start=(first and k == 0), stop=(last and k == NCHUNK - 1))

        # ---- epilogue ----
        ps_sb = one.tile([P, 3 * 384 + 128 + 512], F32)
        for i in range(3):
            nc.vector.tensor_copy(ps_sb[:, i * 384:(i + 1) * 384], ps[i][:])
        nc.vector.tensor_copy(ps_sb[:, 1152:1280], ps0[:])
        nc.vector.tensor_copy(ps_sb[:, 1280:1792], ps4[:])
        nc.sync.dma_start(OUT_ACC[:], acc[:])
        nc.sync.dma_start(OUT_PS[:], ps_sb[:])

    nc.compile()
    return nc


def _get_nc():
    if "nc" not in _NC_CACHE:
        _NC_CACHE["nc"] = _build_nc()
    return _NC_CACHE["nc"]


def _shard_inputs(logits: np.ndarray, target: np.ndarray):
    """Split into 8 per-core input maps; cast to bf16 on host."""
    assert logits.shape == (B, C, D, H, W), logits.shape
    assert target.shape == (B, 1, D, H, W), target.shape
    lg = np.ascontiguousarray(logits).astype(ml_dtypes.bfloat16)
    tg = target[:, 0].astype(np.float32).astype(ml_dtypes.bfloat16)

    d_per_core = D // (NCORES // B)  # 16
    in_maps = []
    for k in range(NCORES):
        b = k // (NCORES // B)
        d0 = (k % (NCORES // B)) * d_per_core
        xs = lg[b, :, d0:d0 + d_per_core].reshape(C, NMT, P, FD)
        ts = tg[b, d0:d0 + d_per_core].reshape(NMT, P, FD)
        in_maps.append({"x": np.ascontiguousarray(xs), "t": np.ascontiguousarray(ts)})
    return in_maps


def _combine(results) -> np.float32:
    ps_sum = np.zeros(3, np.float64)   # p_sum[c], c=1..3
    gt = np.zeros(C, np.float64)       # gt_sum[c], c=0..3
    ca = 0.0                           # sum vf * log s
    xb = np.zeros(C, np.float64)       # sum eq_c * x_c
    inter = np.zeros(3, np.float64)    # intersect[c], c=1..3

    count = 0.0
    for res in results:
        ca += res["out_acc"].astype(np.float64).sum()
        blk = res["out_ps"].astype(np.float64)
        for i in range(3):
            b0 = i * 384
            inter[i] += np.trace(blk[:, b0:b0 + 128])
            xb[i + 1] += np.trace(blk[:, b0 + 128:b0 + 256])
            gt[i + 1] += blk[:, b0 + 256].sum()
        xb[0] += np.trace(blk[:, 1152:1280])
        for c in range(4):
            psc = np.trace(blk[:, 1280 + c * 128:1280 + (c + 1) * 128])
            count += psc
            if c >= 1:
                ps_sum[c - 1] += psc

    ce = (ca - xb.sum()) / count

    gt_fg = gt[1:4]
    denom = ps_sum + gt_fg
    dice = (2.0 * inter + SMOOTH_NR) / (denom + SMOOTH_DR)
    present = (gt_fg > 0).astype(np.float64)
    n_present = present.sum()
    mean_dice = (dice * present).sum() / max(n_present, 1.0)
    dice_loss = (1.0 - mean_dice) if n_present > 0 else 0.0
    return np.float32(dice_loss + ce)


def kernel(logits: np.ndarray, target: np.ndarray) -> np.ndarray:
    nc = _get_nc()
    in_maps = _shard_inputs(np.asarray(logits), np.asarray(target))
    last_exc = None
    for _attempt in range(3):
        try:
            out = run_bass_kernel_spmd(nc, in_maps, core_ids=list(range(NCORES)))
            return _combine(out.results)
        except Exception as exc:  # transient NRT_EXEC_UNIT_UNRECOVERABLE recovers on retry
            last_exc = exc
            import time
            time.sleep(2.0)
    raise last_exc


if __name__ == "__main__":
    rng = np.random.default_rng(0)
    lg = rng.standard_normal((B, C, D, H, W), dtype=np.float32)
    tg = rng.integers(-1, C, (B, 1, D, H, W)).astype(np.int32)
    print(kernel(lg, tg))



# revision 3
# speedup vs baseline: 1.0116x; 1.0116x over previous
"""DiceCE-with-ignore-index loss kernel for Trainium2, 8 NeuronCores.

Contract: kernel(logits, target) -> np.float32 scalar loss, matching
reference: CE (mean over valid voxels) + masked soft Dice (batch dice,
background excluded), ignore_index = -1.

Strategy (v2 — engine-balanced)
-------------------------------
Data-parallel over (b, d): 16 d-slices per core, 1,048,576 voxels each,
processed as NCH=4 chunks of [128, 2048] bf16 planes.  Work is spread
so every engine carries ~50 us:

  ACT    e_c = exp(x_c) (4 planes) ; L = ln s          (5 planes)
  Pool   s01 = e0+e1 ; s23 = e2+e3                     (2 planes)
  DVE    s = s01+s23 ; eq_c = (t==c) with accum_out -> gt_c ;
         vf = (t>=0) with accum_out -> count ;
         r = 1/s via int16 bit trick (one tensor_scalar on the
         bf16 bit pattern: r = bitcast(K - i(s)), K = 32497) ;
         w = vf*r ; z_c = e_c*w ; ca-STT accum -> sum vf*ln s
  PE     diag-trick PSUM accumulation, 5 matmuls per 128-col set:
         pk_c  = eq_c  x [z_c | x_c]  -> intersect[c], xb[c]  (c=1..3)
         ps4   = w     x [e1|e2|e3]   -> p_sum[c]
         ps0   = eq_0  x [x_0]        -> xb[0]

The tiny final combine (traces of PSUM blocks, (C,) vectors) happens on
host in float64.  CE = (sum vf*L - sum_c xb_c) / count.
"""
import os
import sys
from contextlib import ExitStack

for _p in ("/opt/trn_rl_repo", "/root/.axon_site/_ro/trn_rl_repo", "/root/.axon_site"):
    if os.path.isdir(_p) and _p not in sys.path:
        sys.path.append(_p)

import numpy as np
import ml_dtypes

import concourse.bass as bass
import concourse.tile as tile
from concourse import bacc, mybir
from concourse.bass_utils import run_bass_kernel_spmd

BF16 = mybir.dt.bfloat16
I16 = mybir.dt.int16
F32 = mybir.dt.float32
ALU = mybir.AluOpType
ACTF = mybir.ActivationFunctionType

P = 128
FD = 2048        # free dim per chunk
NCH = 4          # chunks per core (4 * 128 * 2048 = 1,048,576 voxels)
NSET = FD // P   # 16 diag sets per chunk
NCORES = 8
C = 4

B, D, H, W = 2, 64, 256, 256
SMOOTH_NR = 1e-05
SMOOTH_DR = 1e-05
RECIP_K = 32497.0   # bf16 bit-trick reciprocal constant (calibrated)

_NC_CACHE = {}


def _patch_act_tables():
    """Force Exp and Ln into the combined natural_log_exp set: one
    ACT_TABLE_LOAD for the whole kernel."""
    import concourse.hw_specs as hw_specs
    if getattr(bacc, "_act_tables_patched", False):
        return
    orig = hw_specs.get_activation_tables

    def patched(arch):
        tables = {k: set(v) for k, v in orig(arch).items()}
        if "natural_log_exp_and_others" in tables:
            for name, fns in tables.items():
                if name != "natural_log_exp_and_others":
                    fns.discard(ACTF.Exp)
                    fns.discard(ACTF.Ln)
        return tables

    hw_specs.get_activation_tables = patched
    bacc.get_activation_tables = patched
    bacc._act_tables_patched = True


def _build_nc():
    _patch_act_tables()
    nc = bacc.Bacc("TRN2", target_bir_lowering=False, debug=False)

    X = nc.dram_tensor("x", [C, NCH, P, FD], BF16, kind="ExternalInput")
    T = nc.dram_tensor("t", [NCH, P, FD], BF16, kind="ExternalInput")
    # accum columns: [gt1 gt2 gt3 count ca] x NCH
    OUT_ACC = nc.dram_tensor("out_acc", [P, 5 * NCH], F32, kind="ExternalOutput")
    # psum blocks: pk1|pk2|pk3 (256 each) | ps4 (384) | ps0 (128)
    OUT_PS = nc.dram_tensor("out_ps", [P, 3 * 256 + 384 + 128], F32, kind="ExternalOutput")

    with tile.TileContext(nc) as tc, ExitStack() as ctx:
        io = ctx.enter_context(tc.tile_pool(name="io", bufs=2))
        wk = ctx.enter_context(tc.tile_pool(name="wk", bufs=2))
        one = ctx.enter_context(tc.tile_pool(name="one", bufs=1))
        psum = ctx.enter_context(tc.tile_pool(name="psum", bufs=1, space="PSUM"))

        acc = one.tile([P, 5 * NCH], F32, name="acc")
        pk = [psum.tile([P, 256], F32, name=f"pk{c}") for c in (1, 2, 3)]
        ps4 = psum.tile([P, 384], F32, name="ps4")
        ps0 = psum.tile([P, 128], F32, name="ps0")

        for k in range(NCH):
            first, last = k == 0, k == NCH - 1
            # ---- inputs: x1..x3 land in plane 1 of the [z|x] pack tiles ----
            t_sb = io.tile([P, FD], BF16, tag="t", name=f"t_{k}")
            x0 = io.tile([P, FD], BF16, tag="x0", name=f"x0_{k}")
            zx = [io.tile([P, 2, FD], BF16, tag=f"zx{c}", name=f"zx{c}_{k}")
                  for c in (1, 2, 3)]
            nc.sync.dma_start(t_sb[:], T[k])
            nc.sync.dma_start(x0[:], X[0, k])
            for i in range(3):
                nc.sync.dma_start(zx[i][:, 1, :], X[i + 1, k])

            # ---- DVE masks (only need t) ----
            eq = [wk.tile([P, FD], BF16, tag=f"eq{c}", name=f"eq{c}_{k}")
                  for c in range(C)]
            nc.vector.tensor_scalar(out=eq[0][:], in0=t_sb[:], scalar1=0.0,
                                    scalar2=None, op0=ALU.is_equal)
            for c in (1, 2, 3):
                nc.vector.tensor_scalar(
                    out=eq[c][:], in0=t_sb[:], scalar1=float(c), scalar2=0.0,
                    op0=ALU.is_equal, op1=ALU.add,
                    accum_out=acc[:, (c - 1) * NCH + k:(c - 1) * NCH + k + 1])
            vf = wk.tile([P, FD], BF16, tag="vf", name=f"vf_{k}")
            nc.vector.tensor_scalar(out=vf[:], in0=t_sb[:], scalar1=0.0,
                                    scalar2=0.0, op0=ALU.is_ge, op1=ALU.add,
                                    accum_out=acc[:, 3 * NCH + k:3 * NCH + k + 1])

            # ---- ACT exponentials ----
            e0 = wk.tile([P, FD], BF16, tag="e0", name=f"e0_{k}")
            E = wk.tile([P, 3, FD], BF16, tag="E", name=f"E_{k}")
            nc.scalar.activation(e0[:], x0[:], ACTF.Exp)
            for i in range(3):
                nc.scalar.activation(E[:, i, :], zx[i][:, 1, :], ACTF.Exp)

            # ---- softmax denominator: Pool partials, DVE final ----
            s01 = wk.tile([P, FD], BF16, tag="s01", name=f"s01_{k}")
            s23 = wk.tile([P, FD], BF16, tag="s23", name=f"s23_{k}")
            s = wk.tile([P, FD], BF16, tag="s", name=f"s_{k}")
            nc.gpsimd.tensor_tensor(s01[:], e0[:], E[:, 0, :], ALU.add)
            nc.gpsimd.tensor_tensor(s23[:], E[:, 1, :], E[:, 2, :], ALU.add)
            nc.vector.tensor_tensor(s[:], s01[:], s23[:], ALU.add)

            # ---- L = ln s (ACT);  r = 1/s via int16 bit trick (DVE) ----
            L = wk.tile([P, FD], BF16, tag="L", name=f"L_{k}")
            nc.scalar.activation(L[:], s[:], ACTF.Ln)
            rI = wk.tile([P, FD], I16, tag="rI", name=f"rI_{k}")
            nc.vector.tensor_scalar(out=rI[:], in0=s[:].bitcast(I16),
                                    scalar1=-1.0, scalar2=RECIP_K,
                                    op0=ALU.mult, op1=ALU.add)

            # ---- w = vf * r ; z_c = e_c * w ----
            w = wk.tile([P, FD], BF16, tag="w", name=f"w_{k}")
            nc.vector.tensor_tensor(w[:], vf[:], rI[:].bitcast(BF16), ALU.mult)
            for i in range(3):
                nc.vector.tensor_tensor(zx[i][:, 0, :], E[:, i, :], w[:], ALU.mult)

            # ---- ca = sum vf * L (STT with accum) ----
            scr = wk.tile([P, FD], BF16, tag="scr", name=f"scr_{k}")
            nc.vector.scalar_tensor_tensor(
                out=scr[:], in0=t_sb[:], scalar=0.0, in1=L[:],
                op0=ALU.is_ge, op1=ALU.mult,
                accum_out=acc[:, 4 * NCH + k:4 * NCH + k + 1])

            # ---- PE diag-trick accumulation ----
            for j in range(NSET):
                sl = slice(j * P, (j + 1) * P)
                st = (first and j == 0)
                sp = (last and j == NSET - 1)
                for i in range(3):
                    nc.tensor.matmul(pk[i][:], eq[i + 1][:, sl], zx[i][:, :, sl],
                                     start=st, stop=sp)
                nc.tensor.matmul(ps4[:], w[:, sl], E[:, :, sl], start=st, stop=sp)
                nc.tensor.matmul(ps0[:], eq[0][:, sl], x0[:, sl], start=st, stop=sp)

        # ---- epilogue ----
        ps_sb = one.tile([P, 3 * 256 + 384 + 128], F32, name="ps_sb")
        for i in range(3):
            nc.vector.tensor_copy(ps_sb[:, i * 256:(i + 1) * 256], pk[i][:])
        nc.vector.tensor_copy(ps_sb[:, 768:1152], ps4[:])
        nc.vector.tensor_copy(ps_sb[:, 1152:1280], ps0[:])
        nc.sync.dma_start(OUT_ACC[:], acc[:])
        nc.sync.dma_start(OUT_PS[:], ps_sb[:])

    nc.compile()
    return nc


def _get_nc():
    if "nc" not in _NC_CACHE:
        _NC_CACHE["nc"] = _build_nc()
    return _NC_CACHE["nc"]


def _shard_inputs(logits: np.ndarray, target: np.ndarray):
    """Split into 8 per-core input maps; cast to bf16 on host."""
    assert logits.shape == (B, C, D, H, W), logits.shape
    assert target.shape == (B, 1, D, H, W), target.shape
    lg = np.ascontiguousarray(logits).astype(ml_dtypes.bfloat16)
    tg = target[:, 0].astype(np.float32).astype(ml_dtypes.bfloat16)

    d_per_core = D // (NCORES // B)  # 16
    in_maps = []
    for k in range(NCORES):
        b = k // (NCORES // B)
        d0 = (k % (NCORES // B)) * d_per_core
        xs = lg[b, :, d0:d0 + d_per_core].reshape(C, NCH, P, FD)
        ts = tg[b, d0:d0 + d_per_core].reshape(NCH, P, FD)
        in_maps.append({"x": np.ascontiguousarray(xs), "t": np.ascontiguousarray(ts)})
    return in_maps


def _combine(results) -> np.float32:
    inter = np.zeros(3, np.float64)
    xb = np.zeros(C, np.float64)
    ps_sum = np.zeros(3, np.float64)
    gt = np.zeros(C, np.float64)
    count = 0.0
    ca = 0.0

    for res in results:
        a = res["out_acc"].astype(np.float64)
        for c in (1, 2, 3):
            gt[c] += a[:, (c - 1) * NCH:c * NCH].sum()
        count += a[:, 3 * NCH:4 * NCH].sum()
        ca += a[:, 4 * NCH:5 * NCH].sum()
        blk = res["out_ps"].astype(np.float64)
        for i in range(3):
            b0 = i * 256
            inter[i] += np.trace(blk[:, b0:b0 + 128])
            xb[i + 1] += np.trace(blk[:, b0 + 128:b0 + 256])
            ps_sum[i] += np.trace(blk[:, 768 + i * 128:768 + (i + 1) * 128])
        xb[0] += np.trace(blk[:, 1152:1280])

    ce = (ca - xb.sum()) / count

    denom = ps_sum + gt[1:4]
    dice = (2.0 * inter + SMOOTH_NR) / (denom + SMOOTH_DR)
    present = (gt[1:4] > 0).astype(np.float64)
    n_present = present.sum()
    mean_dice = (dice * present).sum() / max(n_present, 1.0)
    dice_loss = (1.0 - mean_dice) if n_present > 0 else 0.0
    return np.float32(dice_loss + ce)


def kernel(logits: np.ndarray, target: np.ndarray) -> np.ndarray:
    nc = _get_nc()
    in_maps = _shard_inputs(np.asarray(logits), np.asarray(target))
    last_exc = None
    for _attempt in range(3):
        try:
            out = run_bass_kernel_spmd(nc, in_maps, core_ids=list(range(NCORES)))
            return _combine(out.results)
        except Exception as exc:  # transient NRT_EXEC_UNIT_UNRECOVERABLE recovers on retry
            last_exc = exc
            import time
            time.sleep(2.0)
    raise last_exc


if __name__ == "__main__":
    rng = np.random.default_rng(0)
    lg = rng.standard_normal((B, C, D, H, W), dtype=np.float32)
    tg = rng.integers(-1, C, (B, 1, D, H, W)).astype(np.int32)
    print(kernel(lg, tg))
